# revision 18
# baseline (speedup 1.0000x reference)
"""Trainium2 Bass kernel for nn_CrossAttention_51539607552970.

Sharding: 8 cores = 2 (batch) x 4 (GQA kv-head groups). Each core computes
4 query heads + its single kv head for one batch element, producing a
partial output (its head-group's contribution through wo); the host sums
the 4 partials per batch element (tensor-parallel unshard).

On-device layout is feature-major: the host passes x/c transposed
([hid, tokens]) and pre-cast to bf16 so every matmul contracts the
partition dimension natively at 1 cycle/row.  Scores are computed
transposed ([keys, q]); the softmax denominator is built by pairwise
bf16 accumulation of the exp tiles (Pool+DVE) followed by a single
ones-matmul per head.  Q-projection blocks are interleaved with
attention blocks (attention block ab only needs query block ab) so PE
stays busy while xT streams in on the second DMA queue.  wo partial
sums are DMA'd directly from PSUM to DRAM.
"""

import sys

sys.path.insert(0, "/opt/trn_rl_repo")

import numpy as np

import concourse.bass as bass
import concourse.mybir as mybir
import concourse.tile as tile
from concourse import bacc
from concourse.bass_utils import run_bass_kernel_spmd
from concourse.masks import make_identity

F32 = mybir.dt.float32
F32R = mybir.dt.float32r
BF16 = mybir.dt.bfloat16
AF = mybir.ActivationFunctionType
OP = mybir.AluOpType

# Problem constants (hardcoded per contract).
B, S, L = 2, 2048, 2048
H, KVH, D = 16, 4, 128
HID = H * D
EPS = 1e-6
SCALE = 1.0 / np.sqrt(D)

NH = 4           # query heads per core
P = 128          # partitions
HC = HID // P    # 16 hid chunks
KC = L // P      # 16 key chunks
PB = 512         # projection block width (tokens)
AB = 512         # attention block width (queries)
NPB = S // PB    # 4
NAB = S // AB    # 4

_compiled = None


def _build():
    nc = bacc.Bacc("TRN2", num_devices=8)

    xT = nc.dram_tensor("xT", [HID, S], BF16, kind="ExternalInput")
    cT = nc.dram_tensor("cT", [HID, L], BF16, kind="ExternalInput")
    wq = nc.dram_tensor("wq", [HID, NH * D], BF16, kind="ExternalInput")
    wk = nc.dram_tensor("wk", [HID, D], BF16, kind="ExternalInput")
    wv = nc.dram_tensor("wv", [HID, D], BF16, kind="ExternalInput")
    wo = nc.dram_tensor("wo", [NH * D, HID], BF16, kind="ExternalInput")
    nqw = nc.dram_tensor("nqw", [P, 1], F32, kind="ExternalInput")
    nkw = nc.dram_tensor("nkw", [P, 1], F32, kind="ExternalInput")
    out = nc.dram_tensor("out", [S, HID], F32, kind="ExternalOutput")

    with nc.allow_low_precision(reason="bf16 matmul inputs"), \
         tile.TileContext(nc) as tc:
        with tc.tile_pool(name="consts", bufs=1) as consts, \
             tc.tile_pool(name="weights", bufs=1) as weights, \
             tc.tile_pool(name="ctstream", bufs=2) as ctpool, \
             tc.tile_pool(name="xtstream", bufs=2) as xtpool, \
             tc.tile_pool(name="kv", bufs=1) as kvpool, \
             tc.tile_pool(name="xqt", bufs=1) as xqtpool, \
             tc.tile_pool(name="small", bufs=2) as small, \
             tc.tile_pool(name="esbp", bufs=4) as esbp, \
             tc.tile_pool(name="outp", bufs=4) as outp, \
             tc.tile_pool(name="psum", bufs=1, space="PSUM") as psum:

            # ---- constants ----
            ones_f = consts.tile([P, P], F32)
            nc.vector.memset(ones_f[:], 1.0)
            ones_b = consts.tile([P, P], BF16)
            nc.scalar.copy(ones_b[:], ones_f[:])
            ident = consts.tile([P, P], F32)
            make_identity(nc, ident)
            nqw_sb = consts.tile([P, 1], F32)
            nc.sync.dma_start(nqw_sb[:], nqw[:])
            nkw_sb = consts.tile([P, 1], F32)
            nc.sync.dma_start(nkw_sb[:], nkw[:])
            eps_sb = consts.tile([P, 1], F32)
            nc.vector.memset(eps_sb[:], EPS)

            # ---- resident weights (single multi-descriptor DMA each) ----
            # wk/wv on the sync queue ahead of the cT stream (needed first);
            # wq/wo on the scalar queue so they load in parallel.
            wk_sb = weights.tile([P, HC * D], BF16)
            wv_sb = weights.tile([P, HC * D], BF16)
            nc.sync.dma_start(wk_sb[:].rearrange("p (hc d) -> p hc d", d=D),
                              wk[:].rearrange("(hc p) d -> p hc d", p=P))
            nc.sync.dma_start(wv_sb[:].rearrange("p (hc d) -> p hc d", d=D),
                              wv[:].rearrange("(hc p) d -> p hc d", p=P))
            wq_sb = weights.tile([P, HC * NH * D], BF16)   # 16 chunks x 512
            nc.scalar.dma_start(wq_sb[:].rearrange("p (hc d) -> p hc d", d=512),
                                wq[:].rearrange("(hc p) d -> p hc d", p=P))
            wo_sb = weights.tile([P, NH * HID], BF16)      # 4 head-chunks x 2048
            nc.scalar.dma_start(wo_sb[:].rearrange("p (h m) -> p h m", m=HID),
                                wo[:].rearrange("(h p) m -> p h m", p=P))

            # ---- persistent activations ----
            kT_sb = kvpool.tile([P, L], BF16)              # [D, keys]
            v_sb = kvpool.tile([P, KC * D], BF16)          # kt-th block = [keys(kt), D]
            xqT_list = [xqtpool.tile([P, S], BF16, name=f"xqT{h}") for h in range(NH)]

            # =========== Phase B: K/V projections (stream cT) ===========
            for kcol in range(4):  # 512-wide key column blocks
                ct_all = ctpool.tile([P, HC * 512], BF16, name="ct",
                                     tag="ctstream")
                nc.sync.dma_start(
                    ct_all[:].rearrange("p (hc c) -> p hc c", c=512),
                    cT[:, kcol * 512:(kcol + 1) * 512]
                    .rearrange("(hc p) c -> p hc c", p=P))

                kps = psum.tile([P, 512], F32, name="kps", tag="combo", bufs=4)
                vps = psum.tile([P, 512], F32, name="vps", tag="combo", bufs=4)
                for hc in range(HC):
                    ct_c = ct_all[:, hc * 512:(hc + 1) * 512]
                    nc.tensor.matmul(kps[:], wk_sb[:, hc * D:(hc + 1) * D],
                                     ct_c,
                                     start=(hc == 0), stop=(hc == HC - 1))
                    nc.tensor.matmul(vps[:], wv_sb[:, hc * D:(hc + 1) * D],
                                     ct_c,
                                     start=(hc == 0), stop=(hc == HC - 1))
                vT_sb = small.tile([P, 512], F32, name="vT", tag="vT")
                nc.vector.tensor_copy(vT_sb[:], vps[:])
                # k rmsnorm over D (partition dim): sumsq via ones matmul
                ksq = small.tile([P, 512], BF16, name="ksq", tag="sq")
                nc.scalar.square(ksq[:], kps[:])
                ksum = psum.tile([P, 512], F32, name="ksum", tag="st", bufs=2)
                nc.tensor.matmul(ksum[:], ones_b[:], ksq[:], start=True, stop=True)
                krs = small.tile([P, 512], F32, name="krs", tag="rs")
                nc.scalar.activation(krs[:], ksum[:], AF.Sqrt,
                                     bias=eps_sb[:], scale=1.0 / D)
                krr = small.tile([P, 512], F32, name="krr", tag="rr")
                nc.vector.reciprocal_approx_fast(out=krr[:], in_=krs[:])
                # kT = (kps * nkw) * rsqrt  (fused)
                nc.vector.scalar_tensor_tensor(
                    out=kT_sb[:, kcol * 512:(kcol + 1) * 512], in0=kps[:],
                    scalar=nkw_sb[:], in1=krr[:], op0=OP.mult, op1=OP.mult)
                # transpose 128x128 blocks -> v_sb [keys, D]
                for j in range(4):
                    kt = kcol * 4 + j
                    tp = psum.tile([P, P], F32, name="tp", tag="st", bufs=2)
                    nc.tensor.transpose(tp[:], vT_sb[:, j * P:(j + 1) * P],
                                        ident[:])
                    nc.vector.tensor_copy(v_sb[:, kt * D:(kt + 1) * D], tp[:])

            # =========== Phase A block: Q projection for one 512-token slab ====
            def emit_xt_dmas(pb):
                xt_all = xtpool.tile([P, HC * PB], BF16, name="xt",
                                     tag="xtstream")
                nc.scalar.dma_start(
                    xt_all[:].rearrange("p (hc c) -> p hc c", c=PB),
                    xT[:, pb * PB:(pb + 1) * PB]
                    .rearrange("(hc p) c -> p hc c", p=P))
                return xt_all

            def emit_qproj(pb, xt_all):
                qpss = [psum.tile([P, PB], F32, name=f"qps{h}",
                                  tag="combo", bufs=4) for h in range(NH)]
                for hc in range(HC):
                    for h in range(NH):
                        nc.tensor.matmul(
                            qpss[h][:],
                            wq_sb[:, hc * 512 + h * D: hc * 512 + (h + 1) * D],
                            xt_all[:, hc * PB:(hc + 1) * PB],
                            start=(hc == 0), stop=(hc == HC - 1))
                for h in range(NH):
                    qps = qpss[h]
                    qsq = small.tile([P, PB], BF16, name="qsq", tag="sq")
                    nc.scalar.square(qsq[:], qps[:])
                    qsum = psum.tile([P, PB], F32, name="qsum", tag="st", bufs=2)
                    nc.tensor.matmul(qsum[:], ones_b[:], qsq[:], start=True,
                                     stop=True)
                    qrs = small.tile([P, PB], F32, name="qrs", tag="rs")
                    nc.scalar.activation(qrs[:], qsum[:], AF.Sqrt,
                                         bias=eps_sb[:], scale=1.0 / D)
                    qrr = small.tile([P, PB], F32, name="qrr", tag="rr")
                    nc.vector.reciprocal_approx_fast(out=qrr[:], in_=qrs[:])
                    nc.vector.scalar_tensor_tensor(
                        out=xqT_list[h][:, pb * PB:(pb + 1) * PB], in0=qps[:],
                        scalar=nqw_sb[:], in1=qrr[:], op0=OP.mult, op1=OP.mult)

            # =========== Phase C block: attention + wo for one query slab ====
            def emit_attn(ab):
                q0 = ab * AB
                attn_map = {}
                for hg in range(2):          # head groups of 2 (PSUM budget)
                    hs = [2 * hg, 2 * hg + 1]
                    attps = {h: psum.tile([P, AB], F32, name=f"attps{h}",
                                          tag="combo", bufs=4) for h in hs}
                    sumps = {h: psum.tile([P, AB], F32, name=f"sumps{h}",
                                          tag="combo", bufs=4) for h in hs}
                    # score pairs: two key-chunks per [128, 2*AB] PSUM tile so
                    # one exp covers both (halves ACT instruction count)
                    for kp in range(KC // 2):
                        sts = {}
                        for h in hs:
                            st = psum.tile([P, 2 * AB], F32, name="st",
                                           tag="st", bufs=2)
                            for j in range(2):
                                kt = 2 * kp + j
                                nc.tensor.matmul(
                                    st[:, j * AB:(j + 1) * AB],
                                    kT_sb[:, kt * P:(kt + 1) * P],
                                    xqT_list[h][:, q0:q0 + AB],
                                    start=True, stop=True)
                            sts[h] = st
                        es = {}
                        for h in hs:
                            e = esbp.tile([P, 2 * AB], BF16, name="e", tag="e")
                            nc.scalar.activation(e[:], sts[h][:], AF.Exp)
                            es[h] = e
                        # alternate heads so consecutive matmuls never hit
                        # the same PSUM bank back-to-back (RMW hazard)
                        for j in range(2):
                            kt = 2 * kp + j
                            for h in hs:
                                nc.tensor.matmul(
                                    attps[h][:],
                                    v_sb[:, kt * D:(kt + 1) * D],
                                    es[h][:, j * AB:(j + 1) * AB],
                                    start=(kt == 0),
                                    stop=(kt == KC - 1))
                        for j in range(2):
                            kt = 2 * kp + j
                            for h in hs:
                                nc.tensor.matmul(
                                    sumps[h][:], ones_b[:],
                                    es[h][:, j * AB:(j + 1) * AB],
                                    start=(kt == 0),
                                    stop=(kt == KC - 1))
                    for h in hs:
                        rr = small.tile([P, AB], F32, name="arr", tag="arr")
                        nc.vector.reciprocal_approx_fast(out=rr[:],
                                                         in_=sumps[h][:])
                        attn = small.tile([P, AB], BF16, name="attn",
                                          tag=f"attn{h}", bufs=2)
                        nc.vector.tensor_tensor(
                            out=attn[:], in0=attps[h][:], in1=rr[:],
                            op=OP.mult)
                        attn_map[h] = attn
                # wo: out[q, :] += attn_h^T @ wo_h; drain PSUM straight to HBM
                for qs in range(AB // P):  # 4
                    wops = [psum.tile([P, 512], F32, name=f"wop{ht}",
                                      tag="combo", bufs=4) for ht in range(4)]
                    for h in range(NH):
                        for ht in range(4):
                            nc.tensor.matmul(
                                wops[ht][:],
                                attn_map[h][:, qs * P:(qs + 1) * P],
                                wo_sb[:, h * HID + ht * 512: h * HID + (ht + 1) * 512],
                                start=(h == 0), stop=(h == NH - 1))
                    for ht in range(4):
                        ot = outp.tile([P, 512], F32, name="ot", tag="ot")
                        nc.vector.tensor_copy(ot[:], wops[ht][:])
                        dq = nc.sync if ht % 2 == 0 else nc.scalar
                        dq.dma_start(
                            out[q0 + qs * P: q0 + (qs + 1) * P,
                                ht * 512:(ht + 1) * 512], ot[:])

            # =========== interleaved schedule: A0, C0|A1, C1|A2, ... ===========
            xt_tiles = emit_xt_dmas(0)
            emit_qproj(0, xt_tiles)
            for ab in range(NAB):
                if ab + 1 < NPB:
                    xt_next = emit_xt_dmas(ab + 1)
                emit_attn(ab)
                if ab + 1 < NPB:
                    emit_qproj(ab + 1, xt_next)

    nc.compile()
    return nc


def _get_compiled():
    global _compiled
    if _compiled is None:
        _compiled = _build()
    return _compiled


def _shard_inputs(x, c, wq, wkv, wo, norm_q_w, norm_k_w):
    import ml_dtypes
    bf16 = ml_dtypes.bfloat16

    x = np.asarray(x, np.float32)
    c = np.asarray(c, np.float32)
    wq = np.asarray(wq, np.float32)
    wkv = np.asarray(wkv, np.float32)
    wo = np.asarray(wo, np.float32)
    nqw = (np.asarray(norm_q_w, np.float32) * np.float32(SCALE)).reshape(P, 1)
    nkw = np.asarray(norm_k_w, np.float32).reshape(P, 1).copy()

    xTs = [np.ascontiguousarray(x[b].T).astype(bf16) for b in range(B)]
    cTs = [np.ascontiguousarray(c[b].T).astype(bf16) for b in range(B)]
    in_maps = []
    for core in range(8):
        b, g = core // 4, core % 4
        blk = wkv[:, g * 256:(g + 1) * 256]
        in_maps.append({
            "xT": xTs[b],
            "cT": cTs[b],
            "wq": np.ascontiguousarray(wq[:, g * 512:(g + 1) * 512]).astype(bf16),
            "wk": np.ascontiguousarray(blk[:, 0::2]).astype(bf16),
            "wv": np.ascontiguousarray(blk[:, 1::2]).astype(bf16),
            "wo": np.ascontiguousarray(wo[g * 512:(g + 1) * 512, :]).astype(bf16),
            "nqw": nqw,
            "nkw": nkw,
        })
    return in_maps


def run_sharded(inputs, trace=False, trace_cores=None):
    """Run the SPMD kernel; returns (full_output, BassKernelResults)."""
    nc = _get_compiled()
    in_maps = _shard_inputs(**inputs)
    res = run_bass_kernel_spmd(nc, in_maps, core_ids=list(range(8)),
                               trace=trace, trace_cores=trace_cores)
    parts = [r["out"] for r in res.results]
    full = np.empty((B, S, HID), np.float32)
    for b in range(B):
        full[b] = np.sum(np.stack([parts[4 * b + g] for g in range(4)], 0),
                         axis=0, dtype=np.float64).astype(np.float32)
    return full, res


def kernel(**inputs) -> np.ndarray:
    out, _ = run_sharded(inputs, trace=False)
    return out


# revision 25
# speedup vs baseline: 1.0200x; 1.0200x over previous
"""Trainium2 Bass kernel for nn_CrossAttention_51539607552970.

Sharding: 8 cores = 2 (batch) x 4 (GQA kv-head groups). Each core computes
4 query heads + its single kv head for one batch element, producing a
partial output (its head-group's contribution through wo); the host sums
the 4 partials per batch element (tensor-parallel unshard).

On-device layout is feature-major: the host passes x/c transposed
([hid, tokens]) and pre-cast to bf16 so every matmul contracts the
partition dimension natively at 1 cycle/row.  Scores are computed
transposed ([keys, q]); the softmax denominator is built by pairwise
bf16 accumulation of the exp tiles (Pool+DVE) followed by a single
ones-matmul per head.  Q-projection blocks are interleaved with
attention blocks (attention block ab only needs query block ab) so PE
stays busy while xT streams in on the second DMA queue.  wo partial
sums are DMA'd directly from PSUM to DRAM.
"""

import sys

sys.path.insert(0, "/opt/trn_rl_repo")

import numpy as np

import concourse.bass as bass
import concourse.mybir as mybir
import concourse.tile as tile
from concourse import bacc
from concourse.bass_utils import run_bass_kernel_spmd
from concourse.masks import make_identity

F32 = mybir.dt.float32
F32R = mybir.dt.float32r
BF16 = mybir.dt.bfloat16
AF = mybir.ActivationFunctionType
OP = mybir.AluOpType

# Problem constants (hardcoded per contract).
B, S, L = 2, 2048, 2048
H, KVH, D = 16, 4, 128
HID = H * D
EPS = 1e-6
SCALE = 1.0 / np.sqrt(D)

NH = 4           # query heads per core
P = 128          # partitions
HC = HID // P    # 16 hid chunks
KC = L // P      # 16 key chunks
PB = 512         # projection block width (tokens)
AB = 512         # attention block width (queries)
NPB = S // PB    # 4
NAB = S // AB    # 4

_compiled = None


def _build():
    nc = bacc.Bacc("TRN2", num_devices=8)

    # All inputs are host-packed so that each [128, N] SBUF tile is one
    # fully-contiguous-per-partition DMA (full HBM bandwidth, 1 descriptor
    # per partition row): element [p, blk, hc, c] = orig[hc*128+p, blk*W+c].
    xT = nc.dram_tensor("xT", [P, NPB * HC * PB], BF16, kind="ExternalInput")
    cT = nc.dram_tensor("cT", [P, 4 * HC * 512], BF16, kind="ExternalInput")
    wq = nc.dram_tensor("wq", [P, HC * NH * D], BF16, kind="ExternalInput")
    wk = nc.dram_tensor("wk", [P, HC * D], BF16, kind="ExternalInput")
    wv = nc.dram_tensor("wv", [P, HC * D], BF16, kind="ExternalInput")
    wo = nc.dram_tensor("wo", [P, NH * HID], BF16, kind="ExternalInput")
    nqw = nc.dram_tensor("nqw", [P, 1], F32, kind="ExternalInput")
    nkw = nc.dram_tensor("nkw", [P, 1], F32, kind="ExternalInput")
    out = nc.dram_tensor("out", [S, HID], F32, kind="ExternalOutput")

    with nc.allow_low_precision(reason="bf16 matmul inputs"), \
         tile.TileContext(nc) as tc:
        with tc.tile_pool(name="consts", bufs=1) as consts, \
             tc.tile_pool(name="weights", bufs=1) as weights, \
             tc.tile_pool(name="ctstream", bufs=2) as ctpool, \
             tc.tile_pool(name="xtstream", bufs=2) as xtpool, \
             tc.tile_pool(name="kv", bufs=1) as kvpool, \
             tc.tile_pool(name="xqt", bufs=1) as xqtpool, \
             tc.tile_pool(name="small", bufs=2) as small, \
             tc.tile_pool(name="esbp", bufs=4) as esbp, \
             tc.tile_pool(name="outp", bufs=4) as outp, \
             tc.tile_pool(name="psum", bufs=1, space="PSUM") as psum:

            # ---- constants ----
            ones_f = consts.tile([P, P], F32)
            nc.vector.memset(ones_f[:], 1.0)
            ones_b = consts.tile([P, P], BF16)
            nc.scalar.copy(ones_b[:], ones_f[:])
            ident = consts.tile([P, P], F32)
            make_identity(nc, ident)
            nqw_sb = consts.tile([P, 1], F32)
            nc.sync.dma_start(nqw_sb[:], nqw[:])
            nkw_sb = consts.tile([P, 1], F32)
            nc.sync.dma_start(nkw_sb[:], nkw[:])
            eps_sb = consts.tile([P, 1], F32)
            nc.vector.memset(eps_sb[:], EPS)

            # ---- resident weights (single multi-descriptor DMA each) ----
            # wk/wv on the sync queue ahead of the cT stream (needed first);
            # wq/wo on the scalar queue so they load in parallel.
            wk_sb = weights.tile([P, HC * D], BF16)
            wv_sb = weights.tile([P, HC * D], BF16)
            nc.sync.dma_start(wk_sb[:], wk[:])
            nc.sync.dma_start(wv_sb[:], wv[:])
            wq_sb = weights.tile([P, HC * NH * D], BF16)   # 16 chunks x 512
            nc.scalar.dma_start(wq_sb[:], wq[:])
            wo_sb = weights.tile([P, NH * HID], BF16)      # 4 head-chunks x 2048
            # (wo DMA is emitted after xt0 in the schedule: needed latest)

            # ---- persistent activations ----
            kT_sb = kvpool.tile([P, L], BF16)              # [D, keys]
            v_sb = kvpool.tile([P, KC * D], BF16)          # kt-th block = [keys(kt), D]
            xqT_list = [xqtpool.tile([P, S], BF16, name=f"xqT{h}") for h in range(NH)]

            # =========== Phase B: K/V projections (stream cT) ===========
            CW = HC * 512
            for kcol in range(4):  # 512-wide key column blocks
                ct_all = ctpool.tile([P, CW], BF16, name="ct", tag="ctstream")
                nc.sync.dma_start(ct_all[:],
                                  cT[:, kcol * CW:(kcol + 1) * CW])

                kps = psum.tile([P, 512], F32, name="kps", tag="combo", bufs=4)
                vps = psum.tile([P, 512], F32, name="vps", tag="combo", bufs=4)
                for hc in range(HC):
                    ct_c = ct_all[:, hc * 512:(hc + 1) * 512]
                    nc.tensor.matmul(kps[:], wk_sb[:, hc * D:(hc + 1) * D],
                                     ct_c,
                                     start=(hc == 0), stop=(hc == HC - 1))
                    nc.tensor.matmul(vps[:], wv_sb[:, hc * D:(hc + 1) * D],
                                     ct_c,
                                     start=(hc == 0), stop=(hc == HC - 1))
                vT_sb = small.tile([P, 512], F32, name="vT", tag="vT")
                nc.vector.tensor_copy(vT_sb[:], vps[:])
                # k rmsnorm over D (partition dim): sumsq via ones matmul
                ksq = small.tile([P, 512], BF16, name="ksq", tag="sq")
                nc.scalar.square(ksq[:], kps[:])
                ksum = psum.tile([P, 512], F32, name="ksum", tag="st", bufs=2)
                nc.tensor.matmul(ksum[:], ones_b[:], ksq[:], start=True, stop=True)
                krs = small.tile([P, 512], F32, name="krs", tag="rs")
                nc.scalar.activation(krs[:], ksum[:], AF.Sqrt,
                                     bias=eps_sb[:], scale=1.0 / D)
                krr = small.tile([P, 512], F32, name="krr", tag="rr")
                nc.vector.reciprocal_approx_fast(out=krr[:], in_=krs[:])
                # kT = (kps * nkw) * rsqrt  (fused)
                nc.vector.scalar_tensor_tensor(
                    out=kT_sb[:, kcol * 512:(kcol + 1) * 512], in0=kps[:],
                    scalar=nkw_sb[:], in1=krr[:], op0=OP.mult, op1=OP.mult)
                # transpose 128x128 blocks -> v_sb [keys, D]
                for j in range(4):
                    kt = kcol * 4 + j
                    tp = psum.tile([P, P], F32, name="tp", tag="st", bufs=2)
                    nc.tensor.transpose(tp[:], vT_sb[:, j * P:(j + 1) * P],
                                        ident[:])
                    nc.vector.tensor_copy(v_sb[:, kt * D:(kt + 1) * D], tp[:])

            # =========== Phase A block: Q projection for one 512-token slab ====
            XW = HC * PB

            def emit_xt_dmas(pb):
                xt_all = xtpool.tile([P, XW], BF16, name="xt", tag="xtstream")
                nc.scalar.dma_start(xt_all[:],
                                    xT[:, pb * XW:(pb + 1) * XW])
                return xt_all

            def emit_qproj(pb, xt_all):
                qpss = [psum.tile([P, PB], F32, name=f"qps{h}",
                                  tag="combo", bufs=4) for h in range(NH)]
                for hc in range(HC):
                    for h in range(NH):
                        nc.tensor.matmul(
                            qpss[h][:],
                            wq_sb[:, hc * 512 + h * D: hc * 512 + (h + 1) * D],
                            xt_all[:, hc * PB:(hc + 1) * PB],
                            start=(hc == 0), stop=(hc == HC - 1))
                for h in range(NH):
                    qps = qpss[h]
                    qsq = small.tile([P, PB], BF16, name="qsq", tag="sq")
                    nc.scalar.square(qsq[:], qps[:])
                    qsum = psum.tile([P, PB], F32, name="qsum", tag="st", bufs=2)
                    nc.tensor.matmul(qsum[:], ones_b[:], qsq[:], start=True,
                                     stop=True)
                    qrs = small.tile([P, PB], F32, name="qrs", tag="rs")
                    nc.scalar.activation(qrs[:], qsum[:], AF.Sqrt,
                                         bias=eps_sb[:], scale=1.0 / D)
                    qrr = small.tile([P, PB], F32, name="qrr", tag="rr")
                    nc.vector.reciprocal_approx_fast(out=qrr[:], in_=qrs[:])
                    nc.vector.scalar_tensor_tensor(
                        out=xqT_list[h][:, pb * PB:(pb + 1) * PB], in0=qps[:],
                        scalar=nqw_sb[:], in1=qrr[:], op0=OP.mult, op1=OP.mult)

            # =========== Phase C block: attention + wo for one query slab ====
            def emit_attn(ab):
                q0 = ab * AB
                attn_map = {}
                for hg in range(2):          # head groups of 2 (PSUM budget)
                    hs = [2 * hg, 2 * hg + 1]
                    attps = {h: psum.tile([P, AB], F32, name=f"attps{h}",
                                          tag="combo", bufs=4) for h in hs}
                    sumps = {h: psum.tile([P, AB], F32, name=f"sumps{h}",
                                          tag="combo", bufs=4) for h in hs}
                    # score pairs: two key-chunks per [128, 2*AB] PSUM tile so
                    # one exp covers both (halves ACT instruction count)
                    for kp in range(KC // 2):
                        sts = {}
                        for h in hs:
                            st = psum.tile([P, 2 * AB], F32, name="st",
                                           tag="st", bufs=2)
                            for j in range(2):
                                kt = 2 * kp + j
                                nc.tensor.matmul(
                                    st[:, j * AB:(j + 1) * AB],
                                    kT_sb[:, kt * P:(kt + 1) * P],
                                    xqT_list[h][:, q0:q0 + AB],
                                    start=True, stop=True)
                            sts[h] = st
                        es = {}
                        for h in hs:
                            e = esbp.tile([P, 2 * AB], BF16, name="e", tag="e")
                            nc.scalar.activation(e[:], sts[h][:], AF.Exp)
                            es[h] = e
                        # alternate heads so consecutive matmuls never hit
                        # the same PSUM bank back-to-back (RMW hazard)
                        for j in range(2):
                            kt = 2 * kp + j
                            for h in hs:
                                nc.tensor.matmul(
                                    attps[h][:],
                                    v_sb[:, kt * D:(kt + 1) * D],
                                    es[h][:, j * AB:(j + 1) * AB],
                                    start=(kt == 0),
                                    stop=(kt == KC - 1))
                        for j in range(2):
                            kt = 2 * kp + j
                            for h in hs:
                                nc.tensor.matmul(
                                    sumps[h][:], ones_b[:],
                                    es[h][:, j * AB:(j + 1) * AB],
                                    start=(kt == 0),
                                    stop=(kt == KC - 1))
                    for h in hs:
                        rr = small.tile([P, AB], F32, name="arr", tag="arr")
                        nc.vector.reciprocal_approx_fast(out=rr[:],
                                                         in_=sumps[h][:])
                        attn = small.tile([P, AB], BF16, name="attn",
                                          tag=f"attn{h}", bufs=2)
                        nc.vector.tensor_tensor(
                            out=attn[:], in0=attps[h][:], in1=rr[:],
                            op=OP.mult)
                        attn_map[h] = attn
                # wo: out[q, :] += attn_h^T @ wo_h; drain PSUM straight to HBM
                for qs in range(AB // P):  # 4
                    wops = [psum.tile([P, 512], F32, name=f"wop{ht}",
                                      tag="combo", bufs=4) for ht in range(4)]
                    for h in range(NH):
                        for ht in range(4):
                            nc.tensor.matmul(
                                wops[ht][:],
                                attn_map[h][:, qs * P:(qs + 1) * P],
                                wo_sb[:, h * HID + ht * 512: h * HID + (ht + 1) * 512],
                                start=(h == 0), stop=(h == NH - 1))
                    for ht in range(4):
                        ot = outp.tile([P, 512], F32, name="ot", tag="ot")
                        nc.vector.tensor_copy(ot[:], wops[ht][:])
                        dq = nc.sync if ht % 2 == 0 else nc.scalar
                        dq.dma_start(
                            out[q0 + qs * P: q0 + (qs + 1) * P,
                                ht * 512:(ht + 1) * 512], ot[:])

            # =========== interleaved schedule: A0, C0|A1, C1|A2, ... ===========
            xt_tiles = emit_xt_dmas(0)
            nc.scalar.dma_start(wo_sb[:], wo[:])
            emit_qproj(0, xt_tiles)
            for ab in range(NAB):
                if ab + 1 < NPB:
                    xt_next = emit_xt_dmas(ab + 1)
                emit_attn(ab)
                if ab + 1 < NPB:
                    emit_qproj(ab + 1, xt_next)

    nc.compile()
    return nc


def _get_compiled():
    global _compiled
    if _compiled is None:
        _compiled = _build()
    return _compiled


def _shard_inputs(x, c, wq, wkv, wo, norm_q_w, norm_k_w):
    import ml_dtypes
    bf16 = ml_dtypes.bfloat16

    x = np.asarray(x, np.float32)
    c = np.asarray(c, np.float32)
    wq = np.asarray(wq, np.float32)
    wkv = np.asarray(wkv, np.float32)
    wo = np.asarray(wo, np.float32)
    nqw = (np.asarray(norm_q_w, np.float32) * np.float32(SCALE)).reshape(P, 1)
    nkw = np.asarray(norm_k_w, np.float32).reshape(P, 1).copy()

    def pack_tokens(a):
        # [S, HID] -> [128, NPB*HC*PB]: [p, pb, hc, c] = a[pb*PB+c, hc*128+p]
        v = a.reshape(NPB, PB, HC, P).transpose(3, 0, 2, 1)
        return np.ascontiguousarray(v.reshape(P, NPB * HC * PB)).astype(bf16)

    def pack_w(w):
        # [HID, M] -> [128, HC*M]: [p, hc, m] = w[hc*128+p, m]
        hid, m = w.shape
        v = w.reshape(hid // P, P, m).transpose(1, 0, 2)
        return np.ascontiguousarray(v.reshape(P, hid // P * m)).astype(bf16)

    xTs = [pack_tokens(x[b]) for b in range(B)]
    cTs = [pack_tokens(c[b]) for b in range(B)]
    in_maps = []
    for core in range(8):
        b, g = core // 4, core % 4
        blk = wkv[:, g * 256:(g + 1) * 256]
        in_maps.append({
            "xT": xTs[b],
            "cT": cTs[b],
            "wq": pack_w(np.ascontiguousarray(wq[:, g * 512:(g + 1) * 512])),
            "wk": pack_w(np.ascontiguousarray(blk[:, 0::2])),
            "wv": pack_w(np.ascontiguousarray(blk[:, 1::2])),
            "wo": pack_w(np.ascontiguousarray(wo[g * 512:(g + 1) * 512, :])),
            "nqw": nqw,
            "nkw": nkw,
        })
    return in_maps


def run_sharded(inputs, trace=False, trace_cores=None):
    """Run the SPMD kernel; returns (full_output, BassKernelResults)."""
    nc = _get_compiled()
    in_maps = _shard_inputs(**inputs)
    res = run_bass_kernel_spmd(nc, in_maps, core_ids=list(range(8)),
                               trace=trace, trace_cores=trace_cores)
    parts = [r["out"] for r in res.results]
    full = np.empty((B, S, HID), np.float32)
    for b in range(B):
        full[b] = np.sum(np.stack([parts[4 * b + g] for g in range(4)], 0),
                         axis=0, dtype=np.float64).astype(np.float32)
    return full, res


def kernel(**inputs) -> np.ndarray:
    out, _ = run_sharded(inputs, trace=False)
    return out


# revision 29
# speedup vs baseline: 1.0683x; 1.0474x over previous
"""Trainium2 Bass kernel for nn_CrossAttention_51539607552970.

Sharding: 8 cores = 2 (batch) x 4 (GQA kv-head groups). Each core computes
4 query heads + its single kv head for one batch element, producing a
partial output (its head-group's contribution through wo); the host sums
the 4 partials per batch element (tensor-parallel unshard).

On-device layout is feature-major: the host passes x/c transposed
([hid, tokens]) and pre-cast to bf16 so every matmul contracts the
partition dimension natively at 1 cycle/row.  Scores are computed
transposed ([keys, q]); the softmax denominator is built by pairwise
bf16 accumulation of the exp tiles (Pool+DVE) followed by a single
ones-matmul per head.  Q-projection blocks are interleaved with
attention blocks (attention block ab only needs query block ab) so PE
stays busy while xT streams in on the second DMA queue.  wo partial
sums are DMA'd directly from PSUM to DRAM.
"""

import sys

sys.path.insert(0, "/opt/trn_rl_repo")

import numpy as np

import concourse.bass as bass
import concourse.mybir as mybir
import concourse.tile as tile
from concourse import bacc
from concourse.bass_utils import run_bass_kernel_spmd
from concourse.masks import make_identity

F32 = mybir.dt.float32
F32R = mybir.dt.float32r
BF16 = mybir.dt.bfloat16
AF = mybir.ActivationFunctionType
OP = mybir.AluOpType

# Problem constants (hardcoded per contract).
B, S, L = 2, 2048, 2048
H, KVH, D = 16, 4, 128
HID = H * D
EPS = 1e-6
SCALE = 1.0 / np.sqrt(D)

NH = 4           # query heads per core
P = 128          # partitions
HC = HID // P    # 16 hid chunks
KC = L // P      # 16 key chunks
PB = 512         # projection block width (tokens)
AB = 512         # attention block width (queries)
NPB = S // PB    # 4
NAB = S // AB    # 4

_compiled = None


def _build():
    nc = bacc.Bacc("TRN2", num_devices=8)

    # All inputs are host-packed so that each [128, N] SBUF tile is one
    # fully-contiguous-per-partition DMA (full HBM bandwidth, 1 descriptor
    # per partition row): element [p, blk, hc, c] = orig[hc*128+p, blk*W+c].
    xT = nc.dram_tensor("xT", [P, NPB * HC * PB], BF16, kind="ExternalInput")
    cT = nc.dram_tensor("cT", [P, 4 * HC * 512], BF16, kind="ExternalInput")
    wq = nc.dram_tensor("wq", [P, HC * NH * D], BF16, kind="ExternalInput")
    wk = nc.dram_tensor("wk", [P, HC * D], BF16, kind="ExternalInput")
    wv = nc.dram_tensor("wv", [P, HC * D], BF16, kind="ExternalInput")
    wo = nc.dram_tensor("wo", [P, NH * HID], BF16, kind="ExternalInput")
    nqw = nc.dram_tensor("nqw", [P, 1], F32, kind="ExternalInput")
    nkw = nc.dram_tensor("nkw", [P, 1], F32, kind="ExternalInput")
    out = nc.dram_tensor("out", [S, HID], F32, kind="ExternalOutput")

    with nc.allow_low_precision(reason="bf16 matmul inputs"), \
         tile.TileContext(nc) as tc:
        with tc.tile_pool(name="consts", bufs=1) as consts, \
             tc.tile_pool(name="weights", bufs=1) as weights, \
             tc.tile_pool(name="ctstream", bufs=3) as ctpool, \
             tc.tile_pool(name="xtstream", bufs=2) as xtpool, \
             tc.tile_pool(name="kv", bufs=1) as kvpool, \
             tc.tile_pool(name="xqt", bufs=1) as xqtpool, \
             tc.tile_pool(name="small", bufs=2) as small, \
             tc.tile_pool(name="esbp", bufs=4) as esbp, \
             tc.tile_pool(name="outp", bufs=4) as outp, \
             tc.tile_pool(name="psum", bufs=1, space="PSUM") as psum:

            # ---- constants ----
            ones_f = consts.tile([P, P], F32)
            nc.vector.memset(ones_f[:], 1.0)
            ones_b = consts.tile([P, P], BF16)
            nc.scalar.copy(ones_b[:], ones_f[:])
            ident = consts.tile([P, P], F32)
            make_identity(nc, ident)
            nqw_sb = consts.tile([P, 1], F32)
            nc.sync.dma_start(nqw_sb[:], nqw[:])
            nkw_sb = consts.tile([P, 1], F32)
            nc.sync.dma_start(nkw_sb[:], nkw[:])
            eps_sb = consts.tile([P, 1], F32)
            nc.vector.memset(eps_sb[:], EPS)

            # ---- resident weights (single multi-descriptor DMA each) ----
            # wk/wv on the sync queue ahead of the cT stream (needed first);
            # wq/wo on the scalar queue so they load in parallel.
            # All startup-critical loads go on ONE queue in need-order:
            # aggregate DMA bandwidth is shared, so concurrency on the other
            # queue would delay ct0 (which gates the first matmul).
            wk_sb = weights.tile([P, HC * D], BF16)
            wv_sb = weights.tile([P, HC * D], BF16)
            nc.sync.dma_start(wk_sb[:], wk[:])
            nc.sync.dma_start(wv_sb[:], wv[:])
            wq_sb = weights.tile([P, HC * NH * D], BF16)   # 16 chunks x 512
            wo_sb = weights.tile([P, NH * HID], BF16)      # 4 head-chunks x 2048
            # (wq is emitted mid-phase-B on sync; wo on scalar during C0)

            # ---- persistent activations ----
            kT_sb = kvpool.tile([P, L], BF16)              # [D, keys]
            v_sb = kvpool.tile([P, KC * D], BF16)          # kt-th block = [keys(kt), D]
            xqT_list = [xqtpool.tile([P, S], BF16, name=f"xqT{h}") for h in range(NH)]

            # =========== Phase B: K/V projections (stream cT) ===========
            CW = HC * 512
            ct_tiles_pref = []
            for kcol in range(2):  # prefetch first two key blocks
                t = ctpool.tile([P, CW], BF16, name="ct", tag="ctstream")
                nc.sync.dma_start(t[:], cT[:, kcol * CW:(kcol + 1) * CW])
                ct_tiles_pref.append(t)
            nc.sync.dma_start(wq_sb[:], wq[:])
            for kcol in range(2, 4):
                t = ctpool.tile([P, CW], BF16, name="ct", tag="ctstream")
                nc.sync.dma_start(t[:], cT[:, kcol * CW:(kcol + 1) * CW])
                ct_tiles_pref.append(t)
            for kcol in range(4):  # 512-wide key column blocks
                ct_all = ct_tiles_pref[kcol]

                kps = psum.tile([P, 512], F32, name="kps", tag="combo", bufs=4)
                vps = psum.tile([P, 512], F32, name="vps", tag="combo", bufs=4)
                for hc in range(HC):
                    ct_c = ct_all[:, hc * 512:(hc + 1) * 512]
                    nc.tensor.matmul(kps[:], wk_sb[:, hc * D:(hc + 1) * D],
                                     ct_c,
                                     start=(hc == 0), stop=(hc == HC - 1))
                    nc.tensor.matmul(vps[:], wv_sb[:, hc * D:(hc + 1) * D],
                                     ct_c,
                                     start=(hc == 0), stop=(hc == HC - 1))
                vT_sb = small.tile([P, 512], F32, name="vT", tag="vT")
                nc.vector.tensor_copy(vT_sb[:], vps[:])
                # k rmsnorm over D (partition dim): sumsq via ones matmul
                ksq = small.tile([P, 512], BF16, name="ksq", tag="sq")
                nc.scalar.square(ksq[:], kps[:])
                ksum = psum.tile([P, 512], F32, name="ksum", tag="st", bufs=2)
                nc.tensor.matmul(ksum[:], ones_b[:], ksq[:], start=True, stop=True)
                krs = small.tile([P, 512], F32, name="krs", tag="rs")
                nc.scalar.activation(krs[:], ksum[:], AF.Sqrt,
                                     bias=eps_sb[:], scale=1.0 / D)
                krr = small.tile([P, 512], F32, name="krr", tag="rr")
                nc.vector.reciprocal_approx_fast(out=krr[:], in_=krs[:])
                # kT = (kps * nkw) * rsqrt  (fused)
                nc.vector.scalar_tensor_tensor(
                    out=kT_sb[:, kcol * 512:(kcol + 1) * 512], in0=kps[:],
                    scalar=nkw_sb[:], in1=krr[:], op0=OP.mult, op1=OP.mult)
                # transpose 128x128 blocks -> v_sb [keys, D]
                for j in range(4):
                    kt = kcol * 4 + j
                    tp = psum.tile([P, P], F32, name="tp", tag="st", bufs=2)
                    nc.tensor.transpose(tp[:], vT_sb[:, j * P:(j + 1) * P],
                                        ident[:])
                    nc.vector.tensor_copy(v_sb[:, kt * D:(kt + 1) * D], tp[:])

            # =========== Phase A block: Q projection for one 512-token slab ====
            XW = HC * PB

            def emit_xt_dmas(pb):
                xt_all = xtpool.tile([P, XW], BF16, name="xt", tag="xtstream")
                nc.scalar.dma_start(xt_all[:],
                                    xT[:, pb * XW:(pb + 1) * XW])
                return xt_all

            def emit_qproj(pb, xt_all):
                qpss = [psum.tile([P, PB], F32, name=f"qps{h}",
                                  tag="combo", bufs=4) for h in range(NH)]
                for hc in range(HC):
                    for h in range(NH):
                        nc.tensor.matmul(
                            qpss[h][:],
                            wq_sb[:, hc * 512 + h * D: hc * 512 + (h + 1) * D],
                            xt_all[:, hc * PB:(hc + 1) * PB],
                            start=(hc == 0), stop=(hc == HC - 1))
                for h in range(NH):
                    qps = qpss[h]
                    qsq = small.tile([P, PB], BF16, name="qsq", tag="sq")
                    nc.scalar.square(qsq[:], qps[:])
                    qsum = psum.tile([P, PB], F32, name="qsum", tag="st", bufs=2)
                    nc.tensor.matmul(qsum[:], ones_b[:], qsq[:], start=True,
                                     stop=True)
                    qrs = small.tile([P, PB], F32, name="qrs", tag="rs")
                    nc.scalar.activation(qrs[:], qsum[:], AF.Sqrt,
                                         bias=eps_sb[:], scale=1.0 / D)
                    qrr = small.tile([P, PB], F32, name="qrr", tag="rr")
                    nc.vector.reciprocal_approx_fast(out=qrr[:], in_=qrs[:])
                    nc.vector.scalar_tensor_tensor(
                        out=xqT_list[h][:, pb * PB:(pb + 1) * PB], in0=qps[:],
                        scalar=nqw_sb[:], in1=qrr[:], op0=OP.mult, op1=OP.mult)

            # =========== Phase C block: attention + wo for one query slab ====
            def emit_attn(ab):
                q0 = ab * AB
                attn_map = {}
                for hg in range(2):          # head groups of 2 (PSUM budget)
                    hs = [2 * hg, 2 * hg + 1]
                    attps = {h: psum.tile([P, AB], F32, name=f"attps{h}",
                                          tag="combo", bufs=4) for h in hs}
                    sumps = {h: psum.tile([P, AB], F32, name=f"sumps{h}",
                                          tag="combo", bufs=4) for h in hs}
                    # score pairs: two key-chunks per [128, 2*AB] PSUM tile so
                    # one exp covers both (halves ACT instruction count)
                    for kp in range(KC // 2):
                        sts = {}
                        for h in hs:
                            st = psum.tile([P, 2 * AB], F32, name="st",
                                           tag="st", bufs=2)
                            for j in range(2):
                                kt = 2 * kp + j
                                nc.tensor.matmul(
                                    st[:, j * AB:(j + 1) * AB],
                                    kT_sb[:, kt * P:(kt + 1) * P],
                                    xqT_list[h][:, q0:q0 + AB],
                                    start=True, stop=True)
                            sts[h] = st
                        es = {}
                        for h in hs:
                            e = esbp.tile([P, 2 * AB], BF16, name="e", tag="e")
                            nc.scalar.activation(e[:], sts[h][:], AF.Exp)
                            es[h] = e
                        # alternate heads so consecutive matmuls never hit
                        # the same PSUM bank back-to-back (RMW hazard)
                        for j in range(2):
                            kt = 2 * kp + j
                            for h in hs:
                                nc.tensor.matmul(
                                    attps[h][:],
                                    v_sb[:, kt * D:(kt + 1) * D],
                                    es[h][:, j * AB:(j + 1) * AB],
                                    start=(kt == 0),
                                    stop=(kt == KC - 1))
                        for j in range(2):
                            kt = 2 * kp + j
                            for h in hs:
                                nc.tensor.matmul(
                                    sumps[h][:], ones_b[:],
                                    es[h][:, j * AB:(j + 1) * AB],
                                    start=(kt == 0),
                                    stop=(kt == KC - 1))
                    for h in hs:
                        rr = small.tile([P, AB], F32, name="arr", tag="arr")
                        nc.vector.reciprocal_approx_fast(out=rr[:],
                                                         in_=sumps[h][:])
                        attn = small.tile([P, AB], BF16, name="attn",
                                          tag=f"attn{h}", bufs=2)
                        nc.vector.tensor_tensor(
                            out=attn[:], in0=attps[h][:], in1=rr[:],
                            op=OP.mult)
                        attn_map[h] = attn
                # wo: out[q, :] += attn_h^T @ wo_h; drain PSUM straight to HBM
                for qs in range(AB // P):  # 4
                    wops = [psum.tile([P, 512], F32, name=f"wop{ht}",
                                      tag="combo", bufs=4) for ht in range(4)]
                    for h in range(NH):
                        for ht in range(4):
                            nc.tensor.matmul(
                                wops[ht][:],
                                attn_map[h][:, qs * P:(qs + 1) * P],
                                wo_sb[:, h * HID + ht * 512: h * HID + (ht + 1) * 512],
                                start=(h == 0), stop=(h == NH - 1))
                    for ht in range(4):
                        ot = outp.tile([P, 512], F32, name="ot", tag="ot")
                        nc.vector.tensor_copy(ot[:], wops[ht][:])
                        dq = nc.sync if ht % 2 == 0 else nc.scalar
                        dq.dma_start(
                            out[q0 + qs * P: q0 + (qs + 1) * P,
                                ht * 512:(ht + 1) * 512], ot[:])

            # =========== interleaved schedule: A0, C0|A1, C1|A2, ... ===========
            def emit_xt_sync(pb):  # startup block rides the need-ordered queue
                xt_all = xtpool.tile([P, XW], BF16, name="xt", tag="xtstream")
                nc.sync.dma_start(xt_all[:], xT[:, pb * XW:(pb + 1) * XW])
                return xt_all

            xt_tiles = emit_xt_sync(0)
            emit_qproj(0, xt_tiles)
            for ab in range(NAB):
                if ab + 1 < NPB:
                    xt_next = emit_xt_dmas(ab + 1)
                if ab == 0:
                    nc.scalar.dma_start(wo_sb[:], wo[:])
                emit_attn(ab)
                if ab + 1 < NPB:
                    emit_qproj(ab + 1, xt_next)

    nc.compile()
    return nc


def _get_compiled():
    global _compiled
    if _compiled is None:
        _compiled = _build()
    return _compiled


def _shard_inputs(x, c, wq, wkv, wo, norm_q_w, norm_k_w):
    import ml_dtypes
    bf16 = ml_dtypes.bfloat16

    x = np.asarray(x, np.float32)
    c = np.asarray(c, np.float32)
    wq = np.asarray(wq, np.float32)
    wkv = np.asarray(wkv, np.float32)
    wo = np.asarray(wo, np.float32)
    nqw = (np.asarray(norm_q_w, np.float32) * np.float32(SCALE)).reshape(P, 1)
    nkw = np.asarray(norm_k_w, np.float32).reshape(P, 1).copy()

    def pack_tokens(a):
        # [S, HID] -> [128, NPB*HC*PB]: [p, pb, hc, c] = a[pb*PB+c, hc*128+p]
        v = a.reshape(NPB, PB, HC, P).transpose(3, 0, 2, 1)
        return np.ascontiguousarray(v.reshape(P, NPB * HC * PB)).astype(bf16)

    def pack_w(w):
        # [HID, M] -> [128, HC*M]: [p, hc, m] = w[hc*128+p, m]
        hid, m = w.shape
        v = w.reshape(hid // P, P, m).transpose(1, 0, 2)
        return np.ascontiguousarray(v.reshape(P, hid // P * m)).astype(bf16)

    xTs = [pack_tokens(x[b]) for b in range(B)]
    cTs = [pack_tokens(c[b]) for b in range(B)]
    in_maps = []
    for core in range(8):
        b, g = core // 4, core % 4
        blk = wkv[:, g * 256:(g + 1) * 256]
        in_maps.append({
            "xT": xTs[b],
            "cT": cTs[b],
            "wq": pack_w(np.ascontiguousarray(wq[:, g * 512:(g + 1) * 512])),
            "wk": pack_w(np.ascontiguousarray(blk[:, 0::2])),
            "wv": pack_w(np.ascontiguousarray(blk[:, 1::2])),
            "wo": pack_w(np.ascontiguousarray(wo[g * 512:(g + 1) * 512, :])),
            "nqw": nqw,
            "nkw": nkw,
        })
    return in_maps


def run_sharded(inputs, trace=False, trace_cores=None):
    """Run the SPMD kernel; returns (full_output, BassKernelResults)."""
    nc = _get_compiled()
    in_maps = _shard_inputs(**inputs)
    res = run_bass_kernel_spmd(nc, in_maps, core_ids=list(range(8)),
                               trace=trace, trace_cores=trace_cores)
    parts = [r["out"] for r in res.results]
    full = np.empty((B, S, HID), np.float32)
    for b in range(B):
        full[b] = np.sum(np.stack([parts[4 * b + g] for g in range(4)], 0),
                         axis=0, dtype=np.float64).astype(np.float32)
    return full, res


def kernel(**inputs) -> np.ndarray:
    out, _ = run_sharded(inputs, trace=False)
    return out
